# revision 1
# baseline (speedup 1.0000x reference)
"""Trainium2 Bass kernel for nn_Diff_prop_18425409699925 (GNN message passing).

Math (per batch element b, with x = local_feat[b] reshaped to [n=1024, c=256]):
  xn   = x / ||x||_row
  A    = (xn * diag(W_adj)) @ xn^T                (symmetric; einsum uses only
                                                   the diagonal of W_adj)
  G    = exp(5*A) with diagonal zeroed            (the reference's row-max
                                                   shift cancels exactly in the
                                                   row-normalized mean)
  M    = (G @ x) / rowsum(G)
  diff = (x - M) @ W_aff^T + b_aff                (algebraic reduction of
                                                   rowsum*(xW+b) - (A@x)W^T)
  y    = LeakyReLU(LayerNorm(diff) * gamma + beta, 0.01)

Sharding: data-parallel over batch B=8 -> one batch element per NeuronCore,
weights replicated, no collectives. G (symmetric) is used directly as the
lhsT of the G @ x matmul, avoiding a [1024,1024] transpose.

Matmuls run in float32r (PE reduced-precision fp32: s+8e+11m), which streams
1 column/cycle like bf16 but keeps ~3 more mantissa bits. Operands consumed
by fp32r matmuls must be produced pre-rounded, hence the float32r dtype tags
and the host-side rounding of DMA-fed operands.

global_feat and pos are unused by the reference; accepted and ignored.
"""

import os
import sys

import numpy as np

for _p in ("/opt/trn_rl_repo",):
    if os.path.isdir(_p) and _p not in sys.path:
        sys.path.insert(0, _p)

import concourse.bacc as bacc
import concourse.bass as bass
import concourse.tile as tile
from concourse import mybir
from concourse.bass_utils import run_bass_kernel_spmd

B, T, NN, C = 8, 16, 64, 256
N = T * NN            # 1024 nodes per batch element
P = 128               # partitions
NT = N // P           # 8 n-tiles
CT = C // P           # 2 c-tiles
F32 = mybir.dt.float32
F32R = mybir.dt.float32r
TS = bass.ts

LN_EPS = 1e-5
LEAK = 0.01
DIAG_NEG = -200.0     # added to diagonal of A pre-exp -> exp underflows to 0


def _build_program(diag_one, ln_trivial, use_bf16):
    nc = bacc.Bacc("TRN2", target_bir_lowering=False, debug=False)

    x_d = nc.declare_dram_parameter("x", [N, C], F32R, isOutput=False)
    wdiag_d = nc.declare_dram_parameter("wdiag", [P, CT], F32, isOutput=False)
    wafft_d = nc.declare_dram_parameter("wafft", [C, C], F32R, isOutput=False)
    rows_d = nc.declare_dram_parameter("rows", [1, 3 * C], F32, isOutput=False)
    ident_d = nc.declare_dram_parameter("ident", [P, P], F32, isOutput=False)
    y_d = nc.declare_dram_parameter("y", [N, C], F32, isOutput=True)

    with tile.TileContext(nc) as tc:
        _emit(nc, tc, x_d, wdiag_d, wafft_d, rows_d, ident_d, y_d,
              diag_one, ln_trivial, use_bf16)
    nc.finalize()
    return nc


def _emit(nc, tc, x_d, wdiag_d, wafft_d, rows_d, ident_d, y_d,
          diag_one, ln_trivial, use_bf16):
    MMDT = mybir.dt.bfloat16 if use_bf16 else F32R
    XNDT = mybir.dt.bfloat16 if use_bf16 else F32
    from contextlib import ExitStack

    mult = mybir.AluOpType.mult
    add = mybir.AluOpType.add
    bypass = mybir.AluOpType.bypass
    amax = mybir.AluOpType.max
    subtract = mybir.AluOpType.subtract
    AF = mybir.ActivationFunctionType

    with ExitStack() as ctx:
        sb = ctx.enter_context(tc.tile_pool(name="sb", bufs=1))
        scr = ctx.enter_context(tc.tile_pool(name="scr", bufs=3))
        ps_a = ctx.enter_context(tc.tile_pool(name="ps_a", bufs=2, space="PSUM"))
        ps_t = ctx.enter_context(tc.tile_pool(name="ps_t", bufs=2, space="PSUM"))
        ps_y = ctx.enter_context(tc.tile_pool(name="ps_y", bufs=2, space="PSUM"))

        # ---------------- persistent SBUF tiles ----------------
        X_all = sb.tile([P, NT, C], F32R, tag="X_all", name="X_all")
        Y_all = sb.tile([P, NT, C], F32, tag="Y_all", name="Y_all")
        xnT = [sb.tile([P, N], MMDT, tag=f"xnT{k}", name=f"xnT{k}")
               for k in range(CT)]
        if diag_one:
            xnTs = xnT
        else:
            xnTs = [sb.tile([P, N], MMDT, tag=f"xnTs{k}", name=f"xnTs{k}")
                    for k in range(CT)]
        G = [sb.tile([P, N], MMDT, tag=f"G{i}", name=f"G{i}") for i in range(NT)]
        D = [sb.tile([P, C], F32, tag=f"D{i}", name=f"D{i}") for i in range(NT)]
        DT = [sb.tile([P, N], F32R, tag=f"DT{k}", name=f"DT{k}")
              for k in range(CT)]
        WT = [sb.tile([P, C], F32R, tag=f"WT{k}", name=f"WT{k}")
              for k in range(CT)]
        diffb = [sb.tile([P, C], F32, tag=f"diffb{i}", name=f"diffb{i}")
                 for i in range(NT)]
        if use_bf16:
            Xb_all = sb.tile([P, NT, C], mybir.dt.bfloat16, tag="Xb_all",
                             name="Xb_all")
        else:
            Xb_all = X_all
        wdiag = sb.tile([P, CT], F32, tag="wdiag", name="wdiag")
        ident = sb.tile([P, P], F32, tag="ident", name="ident")
        negeye = sb.tile([P, P], F32, tag="negeye", name="negeye")
        rows = sb.tile([1, 3 * C], F32, tag="rows", name="rows")

        # batched per-row stats, one column per n-tile
        SS = sb.tile([P, NT], F32, tag="SS", name="SS")
        RNO = sb.tile([P, NT], F32, tag="RNO", name="RNO")
        SNO = sb.tile([P, NT], F32, tag="SNO", name="SNO")
        SP = sb.tile([P, NT], F32, tag="SP", name="SP")
        NSP = sb.tile([P, NT], F32, tag="NSP", name="NSP")
        NRS = sb.tile([P, NT], F32, tag="NRS", name="NRS")
        SU = sb.tile([P, NT], F32, tag="SU", name="SU")
        SQ = sb.tile([P, NT], F32, tag="SQ", name="SQ")
        MU = sb.tile([P, NT], F32, tag="MU", name="MU")
        MUSQ = sb.tile([P, NT], F32, tag="MUSQ", name="MUSQ")
        VAR = sb.tile([P, NT], F32, tag="VAR", name="VAR")
        SD = sb.tile([P, NT], F32, tag="SD", name="SD")
        RSTD = sb.tile([P, NT], F32, tag="RSTD", name="RSTD")
        NB = sb.tile([P, NT], F32, tag="NB", name="NB")
        A8 = sb.tile([P, NT], F32, tag="A8", name="A8")
        eps = sb.tile([P, 1], F32, tag="eps", name="eps")

        # ---------------- loads ----------------
        dma_engines = [nc.gpsimd, nc.scalar, nc.sync]
        for i in range(NT):
            dma_engines[i % 3].dma_start(X_all[:, i, :], x_d[TS(i, P), :])
        nc.sync.dma_start(ident[:], ident_d[:])
        nc.sync.dma_start(wdiag[:], wdiag_d[:])
        for k in range(CT):
            nc.sync.dma_start(WT[k][:], wafft_d[TS(k, P), :])
        nc.sync.dma_start(rows[:], rows_d[:])

        nc.vector.memset(eps[:], LN_EPS)
        nc.vector.tensor_scalar_mul(negeye[:], ident[:], DIAG_NEG)
        if use_bf16:
            identb = sb.tile([P, P], mybir.dt.bfloat16, tag="identb",
                             name="identb")
            nc.vector.tensor_copy(identb[:], ident[:])
        else:
            identb = ident

        if ln_trivial:
            b_bc = g_bc = be_bc = None
        else:
            # replicate b_aff / gamma / beta rows across partitions via PE
            # rank-1 broadcast: ones[1,128]^T @ row[1,512]
            b_bc = sb.tile([P, C], F32, tag="b_bc", name="b_bc")
            g_bc = sb.tile([P, C], F32, tag="g_bc", name="g_bc")
            be_bc = sb.tile([P, C], F32, tag="be_bc", name="be_bc")
            ones1 = sb.tile([1, P], F32, tag="ones1", name="ones1")
            nc.vector.memset(ones1[:], 1.0)
            pbg = ps_y.tile([P, 512], F32, tag="py", name="pbg")
            nc.tensor.matmul(pbg[:], ones1[:], rows[:, 0:512],
                             start=True, stop=True)
            nc.vector.tensor_copy(b_bc[:], pbg[:, 0:C])
            nc.vector.tensor_copy(g_bc[:], pbg[:, C:512])
            pbe = ps_y.tile([P, 512], F32, tag="py", name="pbe")
            nc.tensor.matmul(pbe[:, 0:C], ones1[:], rows[:, 2 * C:3 * C],
                             start=True, stop=True)
            nc.vector.tensor_copy(be_bc[:], pbe[:, 0:C])

        # ---------------- phase 1: row-normalize x, build xn^T ----------------
        for i in range(NT):
            sqs = scr.tile([P, C], F32, tag="sqs", name="sqs")
            nc.vector.scalar_tensor_tensor(
                out=sqs[:], in0=X_all[:, i, :], scalar=1.0,
                in1=X_all[:, i, :], op0=bypass, op1=mult,
                accum_out=SS[:, i:i + 1],
            )
            nc.scalar.activation(SNO[:, i:i + 1], SS[:, i:i + 1], AF.Sqrt)
            nc.vector.reciprocal(RNO[:, i:i + 1], SNO[:, i:i + 1])
            if use_bf16:
                nc.vector.tensor_copy(Xb_all[:, i, :], X_all[:, i, :])
            xn = scr.tile([P, C], XNDT, tag="xn", name="xn")
            nc.vector.tensor_scalar_mul(xn[:], X_all[:, i, :], RNO[:, i:i + 1])
            for k in range(CT):
                pt = ps_t.tile([P, P], XNDT, tag="pt", name="pt")
                nc.tensor.transpose(pt[:], xn[:, TS(k, P)], identb[:])
                nc.vector.tensor_copy(xnT[k][:, TS(i, P)], pt[:])
                if not diag_one:
                    nc.vector.tensor_scalar_mul(
                        xnTs[k][:, TS(i, P)], pt[:], wdiag[:, k:k + 1])

        # ---------------- phase 2: A = xnTs^T @ xnT, G = exp(5A) ----------------
        for i in range(NT):
            pa = ps_a.tile([P, N], F32, tag="pa", name="pa")
            for j in range(2):
                for k in range(CT):
                    nc.tensor.matmul(
                        pa[:, TS(j, 512)],
                        xnTs[k][:, TS(i, P)],
                        xnT[k][:, TS(j, 512)],
                        start=(k == 0), stop=(k == CT - 1),
                    )
            c0 = i * P
            nc.vector.tensor_add(pa[:, c0:c0 + P], pa[:, c0:c0 + P], negeye[:])
            nc.scalar.activation(
                G[i][:], pa[:], AF.Exp, scale=5.0, accum_out=SP[:, i:i + 1])
        nc.vector.tensor_scalar_mul(NSP[:], SP[:], -1.0)
        nc.vector.reciprocal(NRS[:], NSP[:])

        # ---------------- phases 3+4 fused per tile: Y = G @ x, D = x - Y/s,
        # diff = D @ W_aff^T (+b), then LayerNorm + LeakyReLU ----------------
        use_prelu = bool(int(os.environ.get("KERNEL_PRELU", "1")))
        y_g = y_d[:].rearrange("(i p) c -> p i c", p=P)
        for i in range(NT):
            py = ps_y.tile([P, 512], F32, tag="py", name="py")
            for k in range(NT):
                nc.tensor.matmul(
                    py[:, 0:C],
                    G[k][:, TS(i, P)],
                    Xb_all[:, k, :],
                    start=(k == 0), stop=(k == NT - 1),
                )
            nc.vector.scalar_tensor_tensor(
                out=D[i][:], in0=py[:, 0:C], scalar=NRS[:, i:i + 1],
                in1=X_all[:, i, :], op0=mult, op1=add,
            )
            for k in range(CT):
                pt = ps_t.tile([P, P], F32, tag="pt", name="pt")
                nc.tensor.transpose(pt[:], D[i][:, TS(k, P)], ident[:])
                nc.vector.tensor_copy(DT[k][:, TS(i, P)], pt[:])
            pd = ps_a.tile([P, N], F32, tag="pa", name="pd")
            for k in range(CT):
                nc.tensor.matmul(
                    pd[:, 0:C],
                    DT[k][:, TS(i, P)],
                    WT[k][:],
                    start=(k == 0), stop=(k == CT - 1),
                )
            if ln_trivial:
                # out = pd*1 + 0; accum_out = rowsum(pd) (op1 = reduce op)
                nc.vector.tensor_scalar(
                    out=diffb[i][:], in0=pd[:, 0:C], scalar1=1.0,
                    scalar2=0.0, op0=mult, op1=add,
                    accum_out=SU[:, i:i + 1],
                )
            else:
                nc.vector.scalar_tensor_tensor(
                    out=diffb[i][:], in0=pd[:, 0:C], scalar=1.0,
                    in1=b_bc[:], op0=bypass, op1=add,
                    accum_out=SU[:, i:i + 1],
                )
            sqo = scr.tile([P, C], F32, tag="sqo", name="sqo")
            nc.vector.scalar_tensor_tensor(
                out=sqo[:], in0=diffb[i][:], scalar=1.0, in1=diffb[i][:],
                op0=bypass, op1=mult, accum_out=SQ[:, i:i + 1],
            )

        # batched LN stats: mu, var = E[d^2]-mu^2, rstd = 1/sqrt(var+eps)
        nc.vector.tensor_scalar_mul(MU[:], SU[:], 1.0 / C)
        nc.vector.tensor_mul(MUSQ[:], MU[:], MU[:])
        nc.vector.scalar_tensor_tensor(
            out=VAR[:], in0=SQ[:], scalar=1.0 / C, in1=MUSQ[:],
            op0=mult, op1=subtract,
        )
        nc.scalar.activation(SD[:], VAR[:], AF.Sqrt, bias=eps[:])
        nc.vector.reciprocal(RSTD[:], SD[:])
        nc.vector.scalar_tensor_tensor(
            out=NB[:], in0=MU[:], scalar=-1.0, in1=RSTD[:],
            op0=mult, op1=mult)

        for i in range(NT):
            t = scr.tile([P, C], F32, tag="t", name="t")
            nc.vector.tensor_scalar(
                out=t[:], in0=diffb[i][:], scalar1=RSTD[:, i:i + 1],
                scalar2=NB[:, i:i + 1], op0=mult, op1=add,
            )
            if not ln_trivial:
                u = scr.tile([P, C], F32, tag="u", name="u")
                nc.vector.tensor_mul(u[:], t[:], g_bc[:])
                v = scr.tile([P, C], F32, tag="v", name="v")
                nc.vector.tensor_add(v[:], u[:], be_bc[:])
                t = v
            # leaky relu, alternating ACT Prelu / DVE max so the finisher
            # chain runs on two engines in parallel
            if use_prelu and i % 2 == 0:
                nc.scalar.activation(Y_all[:, i, :], t[:], AF.Prelu,
                                     alpha=LEAK)
            else:
                nc.vector.scalar_tensor_tensor(
                    out=Y_all[:, i, :], in0=t[:], scalar=LEAK, in1=t[:],
                    op0=mult, op1=amax,
                )
            if i == NT // 2 - 1:
                nc.sync.dma_start(y_g[:, 0:NT // 2, :],
                                  Y_all[:, 0:NT // 2, :])
        nc.sync.dma_start(y_g[:, NT // 2:NT, :], Y_all[:, NT // 2:NT, :])


_PROGRAM_CACHE = {}
last_results = None


def _get_program(diag_one=True, ln_trivial=True, use_bf16=None):
    if use_bf16 is None:
        use_bf16 = bool(int(os.environ.get("KERNEL_BF16", "0")))
    key = (diag_one, ln_trivial, use_bf16)
    if key not in _PROGRAM_CACHE:
        _PROGRAM_CACHE[key] = _build_program(diag_one, ln_trivial, use_bf16)
    return _PROGRAM_CACHE[key]


def _round_f32r(a):
    """Round fp32 -> float32r (s+8e+11m, top 20 bits) round-to-nearest-even,
    matching the PE's reduced-precision operand format."""
    u = np.ascontiguousarray(a, dtype=np.float32).view(np.uint32)
    r = (u + np.uint32(0x7FF) + ((u >> np.uint32(12)) & np.uint32(1))) \
        & np.uint32(0xFFFFF000)
    return r.view(np.float32)


def _prep_inputs(local_feat, W_adj, W_aff, b_aff, ln_gamma, ln_beta):
    local_feat = _round_f32r(np.asarray(local_feat, dtype=np.float32))
    diag = np.ascontiguousarray(np.diagonal(np.asarray(W_adj, np.float32)))
    wd = np.ascontiguousarray(diag.reshape(CT, P).T)
    wafft = _round_f32r(
        np.ascontiguousarray(np.asarray(W_aff, np.float32).T))
    b = np.asarray(b_aff, np.float32).ravel()
    g = np.asarray(ln_gamma, np.float32).ravel()
    be = np.asarray(ln_beta, np.float32).ravel()
    rows = np.concatenate([b, g, be]).reshape(1, 3 * C)
    ident = np.eye(P, dtype=np.float32)
    diag_one = bool(np.all(diag == 1.0))
    ln_trivial = bool(np.all(g == 1.0) and np.all(be == 0.0)
                      and np.all(b == 0.0))
    x_full = local_feat.reshape(B, N, C)
    in_maps = [
        {"x": np.ascontiguousarray(x_full[bb]), "wdiag": wd, "wafft": wafft,
         "rows": rows, "ident": ident}
        for bb in range(B)
    ]
    return in_maps, diag_one, ln_trivial


def kernel(local_feat, global_feat, pos, W_adj, W_aff, b_aff, ln_gamma,
           ln_beta, **_unused):
    global last_results
    in_maps, diag_one, ln_trivial = _prep_inputs(
        local_feat, W_adj, W_aff, b_aff, ln_gamma, ln_beta)
    nc = _get_program(diag_one, ln_trivial)
    trace = bool(int(os.environ.get("KERNEL_TRACE", "0")))
    res = run_bass_kernel_spmd(nc, in_maps, list(range(B)), trace=trace)
    last_results = res
    out = np.stack([res.results[bb]["y"] for bb in range(B)], axis=0)
    return out.reshape(B, T, NN, C)

